# revision 1
# baseline (speedup 1.0000x reference)
"""Trainium2 Bass kernel for causal multi-head attention (B=4, T=2048, D=1024, H=16).

Sharding: 8 cores = 4 batches x 2 head-groups (8 heads each).
Per core pipeline (Tile framework, single SPMD program):
  phase 1: Q/K projections into transposed per-head-pair layout QT/KT [128=2*64, T],
           V projection into [t, 8*65] layout (65th col per head = ones, for rowsums)
  phase 2: per (q-range of 512, head-pair): causal flash attention in transposed
           layout: ST[k,q] = KT-slice^T @ QT-slice (row-packed pair of fp32r matmuls),
           PT = exp(ST) (ACT), causal mask on diagonal tiles (DVE mul),
           OT[hd+1, q] += [V|1]^T @ PT (bf16), normalize by reciprocal rowsum.
  phase 3: output projection YT[dout, t] = Wo_gT^T @ OT (bf16) + bias (g=0 adds bo)
  ReduceScatter(add) across the batch pair -> each core outputs its dout half.
Host: transpose/slice weights, assemble [B, T, D] from per-core [512, T] halves.
"""

import numpy as np

B, T, D, H, HD = 4, 2048, 1024, 16, 64
NCORES = 8
NP = 4          # head pairs per core
NJ = 4          # q-ranges of 512
QW = 512
TB = T // 128   # 16

_CACHE = {}


def _build_nc():
    import concourse.mybir as mybir
    import concourse.tile as tile
    from concourse import bacc

    F32 = mybir.dt.float32
    F32R = mybir.dt.float32r
    BF16 = mybir.dt.bfloat16
    F16 = mybir.dt.float16
    AF = mybir.ActivationFunctionType
    ALU = mybir.AluOpType

    nc = bacc.Bacc(None, target_bir_lowering=False)
    xt_d = nc.declare_dram_parameter("xt", [D, T], BF16, isOutput=False)
    wq_d = nc.declare_dram_parameter("wq", [D, 512], BF16, isOutput=False)
    wk_d = nc.declare_dram_parameter("wk", [D, 512], BF16, isOutput=False)
    wv_d = nc.declare_dram_parameter("wv", [D, 512], BF16, isOutput=False)
    wo_d = nc.declare_dram_parameter("wo", [512, D], BF16, isOutput=False)
    bias_d = nc.declare_dram_parameter("bias", [128, 8], F32, isOutput=False)
    mask_d = nc.declare_dram_parameter("mask", [128, 512], BF16, isOutput=False)
    yt_d = nc.declare_dram_parameter("yt", [512, T], F32, isOutput=True)

    RG = [[0, 1], [2, 3], [4, 5], [6, 7]]

    with tile.TileContext(nc) as tc:
        with (
            tc.tile_pool(name="persist", bufs=1) as pers,
            tc.tile_pool(name="work", bufs=1) as work,
            tc.tile_pool(name="dram", bufs=1, space="DRAM") as dram,
            tc.tile_pool(name="psum", bufs=1, space="PSUM") as psum,
        ):
            qt = pers.tile([128, NP, T], BF16)
            kt = pers.tile([128, NP, T], BF16)
            v = pers.tile([128, TB, 8 * 65], BF16)
            ot = pers.tile([128, NP, T], BF16)
            m0 = pers.tile([128, 512], BF16)
            wo = pers.tile([128, 4, D], BF16)
            bias = pers.tile([128, 8], F32)
            wq = pers.tile([128, 8, 512], BF16)
            wk = pers.tile([128, 8, 512], BF16)
            wv = pers.tile([128, 8, 512], BF16)
            nc.sync.dma_start(out=m0[:], in_=mask_d[:])
            nc.sync.dma_start(out=bias[:], in_=bias_d[:])
            nc.sync.dma_start(out=wo[:], in_=wo_d.rearrange("(c p) n -> p c n", p=128))
            for c in range(8):
                nc.sync.dma_start(out=wq[:, c, :], in_=wq_d[c * 128:(c + 1) * 128, :])
                nc.sync.dma_start(out=wk[:, c, :], in_=wk_d[c * 128:(c + 1) * 128, :])
                nc.sync.dma_start(out=wv[:, c, :], in_=wv_d[c * 128:(c + 1) * 128, :])

            yt_part = dram.tile([NJ, 1024, QW], F32)
            yt_rs = dram.tile([NJ, 512, QW], F32)

            for j in range(NJ):
                # ---------------- phase 1(j): projections for this t-range ----------------
                xsl = []
                for c in range(8):
                    xs = work.tile([128, QW], BF16, tag="xs", bufs=12)
                    nc.sync.dma_start(
                        out=xs[:], in_=xt_d[c * 128:(c + 1) * 128, j * QW:(j + 1) * QW]
                    )
                    xsl.append(xs)
                for p in range(NP):
                    for w_sb, dst in ((wq, qt), (wk, kt)):
                        acc = psum.tile([128, QW], F32, tag="small", bufs=2)
                        for c in range(8):
                            nc.tensor.matmul(
                                acc[:],
                                w_sb[:, c, p * 128:(p + 1) * 128],
                                xsl[c][:],
                                start=(c == 0),
                                stop=(c == 7),
                            )
                        nc.vector.tensor_copy(dst[:, p, j * QW:(j + 1) * QW], acc[:])
                for sub in range(4):
                    i = 4 * j + sub
                    acc = psum.tile([128, QW], F32, tag="small", bufs=2)
                    for c in range(8):
                        nc.tensor.matmul(
                            acc[:],
                            xsl[c][:, sub * 128:(sub + 1) * 128],
                            wv[:, c, :],
                            start=(c == 0),
                            stop=(c == 7),
                        )
                    vblk = v[:, i, :].rearrange("p (h c) -> p h c", c=65)
                    nc.vector.tensor_copy(
                        vblk[:, :, 0:64],
                        acc[:].rearrange("p (h c) -> p h c", c=64),
                    )
                    nc.gpsimd.memset(vblk[:, :, 64:65], 1.0)

                # ---------------- phase 2(j): attention ----------------
                for p in range(NP):
                    hA, hB = 2 * p, 2 * p + 1
                    o_A = psum.tile([65, QW], F32, tag="o", bufs=2)
                    o_B = psum.tile([65, QW], F32, tag="o", bufs=2)
                    nkb = 4 * j + 4
                    for kb in range(nkb):
                        o = kb - 4 * j  # diagonal offset; < 0 means full block
                        lo = 128 * o if o > 0 else 0  # first live q col in range
                        W = QW - lo
                        st = psum.tile([128, 1024], F32, tag="st", bufs=2)
                        kcols = slice(kb * 128, (kb + 1) * 128)
                        qcols = slice(j * QW + lo, (j + 1) * QW)
                        nc.tensor.matmul(
                            st[:, lo:QW],
                            kt[0:64, p, kcols],
                            qt[0:64, p, qcols],
                            start=True, stop=True, tile_position=(0, 0),
                        )
                        nc.tensor.matmul(
                            st[:, QW + lo:2 * QW],
                            kt[64:128, p, kcols],
                            qt[64:128, p, qcols],
                            start=True, stop=True, tile_position=(64, 0),
                        )
                        pt = work.tile([128, 1024], BF16, tag="pt", bufs=3)
                        nc.scalar.activation(
                            pt[:].rearrange("p (h q) -> p h q", h=2)[:, :, lo:QW],
                            st[:].rearrange("p (h q) -> p h q", h=2)[:, :, lo:QW],
                            AF.Exp,
                        )
                        if o >= 0:
                            nc.vector.tensor_mul(pt[:, lo:QW], pt[:, lo:QW], m0[:, 0:W])
                            nc.vector.tensor_mul(
                                pt[:, QW + lo:2 * QW], pt[:, QW + lo:2 * QW], m0[:, 0:W]
                            )
                        nc.tensor.matmul(
                            o_A[:, lo:QW],
                            v[:, kb, hA * 65:(hA + 1) * 65],
                            pt[:, lo:QW],
                            start=(kb == 0), stop=(kb == nkb - 1),
                        )
                        nc.tensor.matmul(
                            o_B[:, lo:QW],
                            v[:, kb, hB * 65:(hB + 1) * 65],
                            pt[:, QW + lo:2 * QW],
                            start=(kb == 0), stop=(kb == nkb - 1),
                        )
                    # normalize: ot[:, p, jrange] = o / rowsum
                    # copy psum accumulators out first so the o slots free early
                    ocp = work.tile([65, 1024], F32, tag="ocp", bufs=3)
                    nc.vector.tensor_copy(ocp[:, 0:QW], o_A[:])
                    nc.vector.tensor_copy(ocp[:, QW:1024], o_B[:])
                    rec = work.tile([1, 1024], F32, tag="rec", bufs=2)
                    nc.vector.reciprocal(rec[:, 0:QW], ocp[64:65, 0:QW])
                    nc.vector.reciprocal(rec[:, QW:1024], ocp[64:65, QW:1024])
                    bc = work.tile([64, 1024], F32, tag="bc", bufs=2)
                    nc.gpsimd.partition_broadcast(bc[:, 0:QW], rec[:, 0:QW], channels=64)
                    nc.gpsimd.partition_broadcast(bc[:, QW:1024], rec[:, QW:1024], channels=64)
                    jr = slice(j * QW, (j + 1) * QW)
                    nc.vector.tensor_mul(ot[0:64, p, jr], ocp[0:64, 0:QW], bc[:, 0:QW])
                    nc.vector.tensor_mul(ot[64:128, p, jr], ocp[0:64, QW:1024], bc[:, QW:1024])

                # ---------------- phase 3(j): output projection + RS ----------------
                jr = slice(j * QW, (j + 1) * QW)
                for n in range(8):
                    yps = psum.tile([128, QW], F32, tag="st", bufs=2)
                    for c in range(4):
                        nc.tensor.matmul(
                            yps[:],
                            wo[:, c, n * 128:(n + 1) * 128],
                            ot[:, c, jr],
                            start=(c == 0), stop=(c == 3),
                        )
                    ysb = work.tile([128, QW], F32, tag="ysb", bufs=3)
                    nc.vector.tensor_scalar_add(ysb[:], yps[:], bias[:, n:n + 1])
                    nc.sync.dma_start(
                        out=yt_part[j, n * 128:(n + 1) * 128, :], in_=ysb[:]
                    )
                nc.gpsimd.collective_compute(
                    "ReduceScatter",
                    ALU.add,
                    replica_groups=RG,
                    ins=[yt_part[j].opt()],
                    outs=[yt_rs[j].opt()],
                )
                nc.sync.dma_start(out=yt_d[:, jr], in_=yt_rs[j])

    nc.finalize()
    return nc


def _prep_inputs(x, Wq, Wk, Wv, Wo, bo):
    """Build the 8 per-core input maps (host-side layout prep only)."""
    import ml_dtypes

    scale = 1.0 / np.sqrt(np.float32(HD))
    kr = np.arange(128, dtype=np.float32)[:, None]
    qc = np.arange(512, dtype=np.float32)[None, :]
    m0 = (qc >= kr).astype(ml_dtypes.bfloat16)

    in_maps = []
    for c in range(NCORES):
        b, g = c // 2, c % 2
        hs = slice(g * 8, (g + 1) * 8)
        xt = np.ascontiguousarray(x[b].T).astype(ml_dtypes.bfloat16)
        wq = np.ascontiguousarray(Wq[hs].reshape(512, D).T * scale).astype(ml_dtypes.bfloat16)
        wk = np.ascontiguousarray(Wk[hs].reshape(512, D).T).astype(ml_dtypes.bfloat16)
        wv = np.ascontiguousarray(Wv[hs].reshape(512, D).T).astype(ml_dtypes.bfloat16)
        wo = np.ascontiguousarray(Wo[:, g * 512:(g + 1) * 512].T).astype(ml_dtypes.bfloat16)
        if g == 0:
            bias = np.ascontiguousarray(bo.reshape(8, 128).T)
        else:
            bias = np.zeros((128, 8), np.float32)
        in_maps.append(
            {"xt": xt, "wq": wq, "wk": wk, "wv": wv, "wo": wo, "bias": bias, "mask": m0}
        )
    return in_maps


def _run(inputs, trace=False, trace_cores=None):
    from concourse.bass_utils import run_bass_kernel_spmd

    if "nc" not in _CACHE:
        _CACHE["nc"] = _build_nc()
    nc = _CACHE["nc"]
    in_maps = _prep_inputs(
        inputs["x"], inputs["Wq"], inputs["Wk"], inputs["Wv"], inputs["Wo"], inputs["bo"]
    )
    r = run_bass_kernel_spmd(
        nc, in_maps, list(range(NCORES)), trace=trace, trace_cores=trace_cores
    )
    y = np.empty((B, T, D), np.float32)
    for b in range(B):
        yt = np.concatenate([r.results[2 * b]["yt"], r.results[2 * b + 1]["yt"]], axis=0)
        y[b] = yt.T.astype(np.float32)
    return y, r


def kernel(**inputs):
    y, _ = _run(inputs, trace=False)
    return y



# revision 3
# speedup vs baseline: 1.0234x; 1.0234x over previous
"""Trainium2 Bass kernel for causal multi-head attention (B=4, T=2048, D=1024, H=16).

Sharding: 8 cores = 4 batches x 2 head-groups (8 heads each).
Per core pipeline (Tile framework, single SPMD program):
  phase 1: Q/K projections into transposed per-head-pair layout QT/KT [128=2*64, T],
           V projection into [t, 8*65] layout (65th col per head = ones, for rowsums)
  phase 2: per (q-range of 512, head-pair): causal flash attention in transposed
           layout: ST[k,q] = KT-slice^T @ QT-slice (row-packed pair of matmuls),
           PT = exp(ST) (ACT), causal triangle mask on diagonal 128x128 sub-blocks
           (DVE mul), OT[hd+1, q] += [V|1]^T @ PT (bf16), normalize by fast
           reciprocal of rowsum read directly from PSUM.
  phase 3: output projection YT[dout, t] = Wo_gT^T @ OT (bf16) + bias (g=0 adds bo)
           2-chunk ReduceScatter(add) across the batch pair (d-blocks ordered
           [0,1,4,5 | 2,3,6,7] so each chunk scatters contiguously per core)
Host: transpose/slice weights, assemble [B, T, D] from per-core [512, T] halves.
"""

import numpy as np

B, T, D, H, HD = 4, 2048, 1024, 16, 64
NCORES = 8
NP = 4          # head pairs per core
NJ = 4          # q-ranges of 512
QW = 512
TB = T // 128   # 16

_CACHE = {}

# d-block emission order: chunk A = {0,1,4,5} (scatters to [0:256|512:768]),
# chunk B = {2,3,6,7}.  Row position of block n inside yt_part:
_NORDER = [0, 1, 4, 5, 2, 3, 6, 7]
_NPOS = {n: i * 128 for i, n in enumerate(_NORDER)}


def _build_nc():
    import concourse.mybir as mybir
    import concourse.tile as tile
    from concourse import bacc

    F32 = mybir.dt.float32
    BF16 = mybir.dt.bfloat16
    AF = mybir.ActivationFunctionType
    ALU = mybir.AluOpType

    nc = bacc.Bacc(None, target_bir_lowering=False)
    xt_d = nc.declare_dram_parameter("xt", [D, T], BF16, isOutput=False)
    wq_d = nc.declare_dram_parameter("wq", [D, 512], BF16, isOutput=False)
    wk_d = nc.declare_dram_parameter("wk", [D, 512], BF16, isOutput=False)
    wv_d = nc.declare_dram_parameter("wv", [D, 512], BF16, isOutput=False)
    wo_d = nc.declare_dram_parameter("wo", [512, D], BF16, isOutput=False)
    bias_d = nc.declare_dram_parameter("bias", [128, 8], F32, isOutput=False)
    mask_d = nc.declare_dram_parameter("mask", [128, 128], BF16, isOutput=False)
    yt_d = nc.declare_dram_parameter("yt", [512, T], F32, isOutput=True)

    RG = [[0, 1], [2, 3], [4, 5], [6, 7]]

    with tile.TileContext(nc) as tc:
        with (
            tc.tile_pool(name="persist", bufs=1) as pers,
            tc.tile_pool(name="work", bufs=1) as work,
            tc.tile_pool(name="dram", bufs=1, space="DRAM") as dram,
            tc.tile_pool(name="psum", bufs=1, space="PSUM") as psum,
        ):
            qt = pers.tile([128, NP, T], BF16)
            kt = pers.tile([128, NP, T], BF16)
            v = pers.tile([128, TB, 8 * 65], BF16)
            ot = pers.tile([128, NP, T], BF16)
            m0 = pers.tile([128, 128], BF16)
            wo = pers.tile([128, 4, D], BF16)
            bias = pers.tile([128, 8], F32)
            wq = pers.tile([128, 8, 512], BF16)
            wk = pers.tile([128, 8, 512], BF16)
            wv = pers.tile([128, 8, 512], BF16)

            # DMA order tuned for fast ramp: what phase-1(j=0) needs first.
            nc.sync.dma_start(out=m0[:], in_=mask_d[:])
            for c in range(8):
                nc.sync.dma_start(out=wq[:, c, :], in_=wq_d[c * 128:(c + 1) * 128, :])
            xsl0 = []
            for c in range(8):
                xs = work.tile([128, QW], BF16, tag="xs", bufs=12)
                nc.sync.dma_start(out=xs[:], in_=xt_d[c * 128:(c + 1) * 128, 0:QW])
                xsl0.append(xs)
            for c in range(8):
                nc.sync.dma_start(out=wk[:, c, :], in_=wk_d[c * 128:(c + 1) * 128, :])
            for c in range(8):
                nc.sync.dma_start(out=wv[:, c, :], in_=wv_d[c * 128:(c + 1) * 128, :])
            nc.sync.dma_start(out=wo[:], in_=wo_d.rearrange("(c p) n -> p c n", p=128))
            nc.sync.dma_start(out=bias[:], in_=bias_d[:])

            yt_part = dram.tile([NJ, 1024, QW], F32)
            yt_rs = dram.tile([NJ, 512, QW], F32)

            for j in range(NJ):
                # ---------------- phase 1(j): projections for this t-range ----------------
                if j == 0:
                    xsl = xsl0
                else:
                    xsl = []
                    for c in range(8):
                        xs = work.tile([128, QW], BF16, tag="xs", bufs=12)
                        nc.sync.dma_start(
                            out=xs[:], in_=xt_d[c * 128:(c + 1) * 128, j * QW:(j + 1) * QW]
                        )
                        xsl.append(xs)
                for p in range(NP):
                    for w_sb, dst in ((wq, qt), (wk, kt)):
                        acc = psum.tile([128, QW], F32, tag="small", bufs=2)
                        for c in range(8):
                            nc.tensor.matmul(
                                acc[:],
                                w_sb[:, c, p * 128:(p + 1) * 128],
                                xsl[c][:],
                                start=(c == 0),
                                stop=(c == 7),
                            )
                        nc.vector.tensor_copy(dst[:, p, j * QW:(j + 1) * QW], acc[:])
                for sub in range(4):
                    i = 4 * j + sub
                    acc = psum.tile([128, QW], F32, tag="small", bufs=2)
                    for c in range(8):
                        nc.tensor.matmul(
                            acc[:],
                            xsl[c][:, sub * 128:(sub + 1) * 128],
                            wv[:, c, :],
                            start=(c == 0),
                            stop=(c == 7),
                        )
                    vblk = v[:, i, :].rearrange("p (h c) -> p h c", c=65)
                    nc.vector.tensor_copy(
                        vblk[:, :, 0:64],
                        acc[:].rearrange("p (h c) -> p h c", c=64),
                    )
                    nc.gpsimd.memset(vblk[:, :, 64:65], 1.0)

                # ---------------- phase 2(j): attention ----------------
                for p in range(NP):
                    hA, hB = 2 * p, 2 * p + 1
                    o_A = psum.tile([65, QW], F32, tag="o", bufs=2)
                    o_B = psum.tile([65, QW], F32, tag="o", bufs=2)
                    nkb = 4 * j + 4
                    for kb in range(nkb):
                        o = kb - 4 * j  # diagonal offset; < 0 means full block
                        lo = 128 * o if o > 0 else 0  # first live q col in range
                        st = psum.tile([128, 1024], F32, tag="st", bufs=2)
                        kcols = slice(kb * 128, (kb + 1) * 128)
                        qcols = slice(j * QW + lo, (j + 1) * QW)
                        nc.tensor.matmul(
                            st[:, lo:QW],
                            kt[0:64, p, kcols],
                            qt[0:64, p, qcols],
                            start=True, stop=True, tile_position=(0, 0),
                        )
                        nc.tensor.matmul(
                            st[:, QW + lo:2 * QW],
                            kt[64:128, p, kcols],
                            qt[64:128, p, qcols],
                            start=True, stop=True, tile_position=(64, 0),
                        )
                        pt = work.tile([128, 1024], BF16, tag="pt", bufs=4)
                        nc.scalar.activation(
                            pt[:].rearrange("p (h q) -> p h q", h=2)[:, :, lo:QW],
                            st[:].rearrange("p (h q) -> p h q", h=2)[:, :, lo:QW],
                            AF.Exp,
                        )
                        if o >= 0:
                            # causal mask: only the first 128 live q cols form a
                            # triangle vs these 128 k rows; the rest are all-keep
                            nc.vector.tensor_mul(
                                pt[:, lo:lo + 128], pt[:, lo:lo + 128], m0[:]
                            )
                            nc.vector.tensor_mul(
                                pt[:, QW + lo:QW + lo + 128],
                                pt[:, QW + lo:QW + lo + 128],
                                m0[:],
                            )
                        nc.tensor.matmul(
                            o_A[:, lo:QW],
                            v[:, kb, hA * 65:(hA + 1) * 65],
                            pt[:, lo:QW],
                            start=(kb == 0), stop=(kb == nkb - 1),
                        )
                        nc.tensor.matmul(
                            o_B[:, lo:QW],
                            v[:, kb, hB * 65:(hB + 1) * 65],
                            pt[:, QW + lo:2 * QW],
                            start=(kb == 0), stop=(kb == nkb - 1),
                        )
                    # normalize: ot[:, p, jrange] = o / rowsum, reading o from
                    # PSUM directly (no staging copy); fast approx reciprocal
                    rsum = work.tile([1, 1024], F32, tag="rsum", bufs=2)
                    nc.vector.tensor_copy(rsum[:, 0:QW], o_A[64:65, :])
                    nc.vector.tensor_copy(rsum[:, QW:1024], o_B[64:65, :])
                    rec = work.tile([1, 1024], F32, tag="rec", bufs=2)
                    nc.vector.reciprocal_approx_fast(rec[:], rsum[:])
                    bc = work.tile([64, 1024], F32, tag="bc", bufs=2)
                    nc.gpsimd.partition_broadcast(bc[:, 0:QW], rec[:, 0:QW], channels=64)
                    nc.gpsimd.partition_broadcast(bc[:, QW:1024], rec[:, QW:1024], channels=64)
                    jr = slice(j * QW, (j + 1) * QW)
                    nc.vector.tensor_mul(ot[0:64, p, jr], o_A[0:64, :], bc[:, 0:QW])
                    nc.vector.tensor_mul(ot[64:128, p, jr], o_B[0:64, :], bc[:, QW:1024])

                # ---------------- phase 3(j): output projection + RS ----------------
                jr = slice(j * QW, (j + 1) * QW)
                for ni, n in enumerate(_NORDER):
                    yps = psum.tile([128, QW], F32, tag="st", bufs=2)
                    for c in range(4):
                        nc.tensor.matmul(
                            yps[:],
                            wo[:, c, n * 128:(n + 1) * 128],
                            ot[:, c, jr],
                            start=(c == 0), stop=(c == 3),
                        )
                    ysb = work.tile([128, QW], F32, tag="ysb", bufs=3)
                    nc.vector.tensor_scalar_add(ysb[:], yps[:], bias[:, n:n + 1])
                    pos = _NPOS[n]
                    nc.sync.dma_start(
                        out=yt_part[j, pos:pos + 128, :], in_=ysb[:]
                    )
                    if ni == 3 or ni == 7:
                        half = slice(0, 512) if ni == 3 else slice(512, 1024)
                        rhalf = slice(0, 256) if ni == 3 else slice(256, 512)
                        nc.gpsimd.collective_compute(
                            "ReduceScatter",
                            ALU.add,
                            replica_groups=RG,
                            ins=[yt_part[j, half, :].opt()],
                            outs=[yt_rs[j, rhalf, :].opt()],
                        )
                        nc.sync.dma_start(
                            out=yt_d[rhalf, jr], in_=yt_rs[j, rhalf, :]
                        )

    nc.finalize()
    return nc


def _prep_inputs(x, Wq, Wk, Wv, Wo, bo):
    """Build the 8 per-core input maps (host-side layout prep only)."""
    import ml_dtypes

    scale = 1.0 / np.sqrt(np.float32(HD))
    kr = np.arange(128, dtype=np.float32)[:, None]
    qc = np.arange(128, dtype=np.float32)[None, :]
    m0 = (qc >= kr).astype(ml_dtypes.bfloat16)

    in_maps = []
    for c in range(NCORES):
        b, g = c // 2, c % 2
        hs = slice(g * 8, (g + 1) * 8)
        xt = np.ascontiguousarray(x[b].T).astype(ml_dtypes.bfloat16)
        wq = np.ascontiguousarray(Wq[hs].reshape(512, D).T * scale).astype(ml_dtypes.bfloat16)
        wk = np.ascontiguousarray(Wk[hs].reshape(512, D).T).astype(ml_dtypes.bfloat16)
        wv = np.ascontiguousarray(Wv[hs].reshape(512, D).T).astype(ml_dtypes.bfloat16)
        wo = np.ascontiguousarray(Wo[:, g * 512:(g + 1) * 512].T).astype(ml_dtypes.bfloat16)
        if g == 0:
            bias = np.ascontiguousarray(bo.reshape(8, 128).T)
        else:
            bias = np.zeros((128, 8), np.float32)
        in_maps.append(
            {"xt": xt, "wq": wq, "wk": wk, "wv": wv, "wo": wo, "bias": bias, "mask": m0}
        )
    return in_maps


def _run(inputs, trace=False, trace_cores=None):
    from concourse.bass_utils import run_bass_kernel_spmd

    if "nc" not in _CACHE:
        _CACHE["nc"] = _build_nc()
    nc = _CACHE["nc"]
    in_maps = _prep_inputs(
        inputs["x"], inputs["Wq"], inputs["Wk"], inputs["Wv"], inputs["Wo"], inputs["bo"]
    )
    r = run_bass_kernel_spmd(
        nc, in_maps, list(range(NCORES)), trace=trace, trace_cores=trace_cores
    )
    y = np.empty((B, T, D), np.float32)
    for b in range(B):
        yt = np.concatenate([r.results[2 * b]["yt"], r.results[2 * b + 1]["yt"]], axis=0)
        y[b] = yt.T.astype(np.float32)
    return y, r


def kernel(**inputs):
    y, _ = _run(inputs, trace=False)
    return y


# revision 5
# speedup vs baseline: 1.0245x; 1.0012x over previous
"""Trainium2 Bass kernel for causal multi-head attention (B=4, T=2048, D=1024, H=16).

Sharding: 8 cores = 4 batches x 2 head-groups (8 heads each).
Per core pipeline (Tile framework, single SPMD program):
  phase 1: Q/K projections into transposed per-head-pair layout QT/KT [128=2*64, T],
           V projection into [t, 8*65] layout (65th col per head = ones, for rowsums)
  phase 2: per (q-range of 512, head-pair): causal flash attention in transposed
           layout: ST[k,q] = KT-slice^T @ QT-slice (row-packed pair of matmuls),
           PT = exp(ST) (ACT), causal triangle mask on diagonal 128x128 sub-blocks
           (DVE mul), OT[hd+1, q] += [V|1]^T @ PT (bf16), normalize by fast
           reciprocal of rowsum.  xs for range j+1 prefetched at phase-2 start.
  phase 3: output projection YT[dout, t] = Wo_gT^T @ OT (bf16) + bias (g=0 adds bo)
           4-chunk bf16 ReduceScatter(add) across the batch pair; chunk i carries
           d-blocks (i, i+4) so each chunk scatters contiguously per core.
Host: transpose/slice weights, assemble [B, T, D] from per-core [512, T] halves.
"""

import numpy as np

B, T, D, H, HD = 4, 2048, 1024, 16, 64
NCORES = 8
NP = 4          # head pairs per core
NJ = 4          # q-ranges of 512
QW = 512
TB = T // 128   # 16

_CACHE = {}


def _build_nc():
    import concourse.mybir as mybir
    import concourse.tile as tile
    from concourse import bacc

    F32 = mybir.dt.float32
    BF16 = mybir.dt.bfloat16
    AF = mybir.ActivationFunctionType
    ALU = mybir.AluOpType

    nc = bacc.Bacc(None, target_bir_lowering=False)
    xt_d = nc.declare_dram_parameter("xt", [D, T], BF16, isOutput=False)
    wq_d = nc.declare_dram_parameter("wq", [D, 512], BF16, isOutput=False)
    wk_d = nc.declare_dram_parameter("wk", [D, 512], BF16, isOutput=False)
    wv_d = nc.declare_dram_parameter("wv", [D, 512], BF16, isOutput=False)
    wo_d = nc.declare_dram_parameter("wo", [512, D], BF16, isOutput=False)
    bias_d = nc.declare_dram_parameter("bias", [128, 8], F32, isOutput=False)
    mask_d = nc.declare_dram_parameter("mask", [128, 128], BF16, isOutput=False)
    yt_d = nc.declare_dram_parameter("yt", [512, T], BF16, isOutput=True)

    RG = [[0, 1], [2, 3], [4, 5], [6, 7]]

    with tile.TileContext(nc) as tc:
        with (
            tc.tile_pool(name="persist", bufs=1) as pers,
            tc.tile_pool(name="work", bufs=1) as work,
            tc.tile_pool(name="dram", bufs=1, space="DRAM") as dram,
            tc.tile_pool(name="psum", bufs=1, space="PSUM") as psum,
        ):
            qt = pers.tile([128, NP, T], BF16)
            kt = pers.tile([128, NP, T], BF16)
            v = pers.tile([128, TB, 8 * 65], BF16)
            ot = pers.tile([128, NP, T], BF16)
            m0 = pers.tile([128, 128], BF16)
            wo = pers.tile([128, 4, D], BF16)
            bias = pers.tile([128, 8], F32)
            wq = pers.tile([128, 8, 512], BF16)
            wk = pers.tile([128, 8, 512], BF16)
            wv = pers.tile([128, 8, 512], BF16)

            def load_xs(j):
                xs = work.tile([128, 8, QW], BF16, tag="xs", bufs=2)
                nc.sync.dma_start(
                    out=xs[:],
                    in_=xt_d[:, j * QW:(j + 1) * QW].rearrange("(c p) t -> p c t", p=128),
                )
                return xs

            # DMA order tuned for fast ramp: what phase-1(j=0) needs first.
            nc.sync.dma_start(out=m0[:], in_=mask_d[:])
            nc.sync.dma_start(out=wq[:], in_=wq_d.rearrange("(c p) n -> p c n", p=128))
            xs_cur = load_xs(0)
            nc.sync.dma_start(out=wk[:], in_=wk_d.rearrange("(c p) n -> p c n", p=128))
            nc.sync.dma_start(out=wv[:], in_=wv_d.rearrange("(c p) n -> p c n", p=128))
            nc.sync.dma_start(out=wo[:], in_=wo_d.rearrange("(c p) n -> p c n", p=128))
            nc.sync.dma_start(out=bias[:], in_=bias_d[:])

            yt_part = dram.tile([NJ, 1024, QW], BF16)
            yt_rs = dram.tile([NJ, 512, QW], BF16)

            for j in range(NJ):
                # ---------------- phase 1(j): projections for this t-range ----------------
                for p in range(NP):
                    for w_sb, dst in ((wq, qt), (wk, kt)):
                        acc = psum.tile([128, QW], F32, tag="small", bufs=2)
                        for c in range(8):
                            nc.tensor.matmul(
                                acc[:],
                                w_sb[:, c, p * 128:(p + 1) * 128],
                                xs_cur[:, c, :],
                                start=(c == 0),
                                stop=(c == 7),
                            )
                        nc.vector.tensor_copy(dst[:, p, j * QW:(j + 1) * QW], acc[:])
                for sub in range(4):
                    i = 4 * j + sub
                    acc = psum.tile([128, QW], F32, tag="small", bufs=2)
                    for c in range(8):
                        nc.tensor.matmul(
                            acc[:],
                            xs_cur[:, c, sub * 128:(sub + 1) * 128],
                            wv[:, c, :],
                            start=(c == 0),
                            stop=(c == 7),
                        )
                    vblk = v[:, i, :].rearrange("p (h c) -> p h c", c=65)
                    nc.vector.tensor_copy(
                        vblk[:, :, 0:64],
                        acc[:].rearrange("p (h c) -> p h c", c=64),
                    )
                    nc.gpsimd.memset(vblk[:, :, 64:65], 1.0)

                # prefetch next t-range's inputs before output DMAs enter the queue
                if j + 1 < NJ:
                    xs_cur = load_xs(j + 1)

                # ---------------- phase 2(j): attention ----------------
                for p in range(NP):
                    hA, hB = 2 * p, 2 * p + 1
                    o_A = psum.tile([65, QW], F32, tag="o", bufs=2)
                    o_B = psum.tile([65, QW], F32, tag="o", bufs=2)
                    nkb = 4 * j + 4
                    for kb in range(nkb):
                        o = kb - 4 * j  # diagonal offset; < 0 means full block
                        lo = 128 * o if o > 0 else 0  # first live q col in range
                        st = psum.tile([128, 1024], F32, tag="st", bufs=2)
                        kcols = slice(kb * 128, (kb + 1) * 128)
                        qcols = slice(j * QW + lo, (j + 1) * QW)
                        nc.tensor.matmul(
                            st[:, lo:QW],
                            kt[0:64, p, kcols],
                            qt[0:64, p, qcols],
                            start=True, stop=True, tile_position=(0, 0),
                        )
                        nc.tensor.matmul(
                            st[:, QW + lo:2 * QW],
                            kt[64:128, p, kcols],
                            qt[64:128, p, qcols],
                            start=True, stop=True, tile_position=(64, 0),
                        )
                        pt = work.tile([128, 1024], BF16, tag="pt", bufs=4)
                        nc.scalar.activation(
                            pt[:].rearrange("p (h q) -> p h q", h=2)[:, :, lo:QW],
                            st[:].rearrange("p (h q) -> p h q", h=2)[:, :, lo:QW],
                            AF.Exp,
                        )
                        if o >= 0:
                            # causal mask: only the first 128 live q cols form a
                            # triangle vs these 128 k rows; the rest are all-keep
                            nc.vector.tensor_mul(
                                pt[:, lo:lo + 128], pt[:, lo:lo + 128], m0[:]
                            )
                            nc.vector.tensor_mul(
                                pt[:, QW + lo:QW + lo + 128],
                                pt[:, QW + lo:QW + lo + 128],
                                m0[:],
                            )
                        nc.tensor.matmul(
                            o_A[:, lo:QW],
                            v[:, kb, hA * 65:(hA + 1) * 65],
                            pt[:, lo:QW],
                            start=(kb == 0), stop=(kb == nkb - 1),
                        )
                        nc.tensor.matmul(
                            o_B[:, lo:QW],
                            v[:, kb, hB * 65:(hB + 1) * 65],
                            pt[:, QW + lo:2 * QW],
                            start=(kb == 0), stop=(kb == nkb - 1),
                        )
                    # normalize: ot[:, p, jrange] = o / rowsum, reading o from
                    # PSUM; rowsum staged to SBUF (custom DVE op needs SBUF src)
                    rsum = work.tile([1, 1024], F32, tag="rsum", bufs=2)
                    nc.vector.tensor_copy(rsum[:, 0:QW], o_A[64:65, :])
                    nc.vector.tensor_copy(rsum[:, QW:1024], o_B[64:65, :])
                    rec = work.tile([1, 1024], F32, tag="rec", bufs=2)
                    nc.vector.reciprocal_approx_fast(rec[:], rsum[:])
                    bc = work.tile([64, 1024], F32, tag="bc", bufs=2)
                    nc.gpsimd.partition_broadcast(bc[:, 0:QW], rec[:, 0:QW], channels=64)
                    nc.gpsimd.partition_broadcast(bc[:, QW:1024], rec[:, QW:1024], channels=64)
                    jr = slice(j * QW, (j + 1) * QW)
                    nc.vector.tensor_mul(ot[0:64, p, jr], o_A[0:64, :], bc[:, 0:QW])
                    nc.vector.tensor_mul(ot[64:128, p, jr], o_B[0:64, :], bc[:, QW:1024])

                # ---------------- phase 3(j): output projection + chunked RS ----------------
                jr = slice(j * QW, (j + 1) * QW)
                for i in range(4):
                    ysb = work.tile([128, 2, QW], BF16, tag="ysb", bufs=3)
                    for s, n in enumerate((i, i + 4)):
                        yps = psum.tile([128, QW], F32, tag="st", bufs=2)
                        for c in range(4):
                            nc.tensor.matmul(
                                yps[:],
                                wo[:, c, n * 128:(n + 1) * 128],
                                ot[:, c, jr],
                                start=(c == 0), stop=(c == 3),
                            )
                        nc.vector.tensor_scalar_add(ysb[:, s, :], yps[:], bias[:, n:n + 1])
                    nc.sync.dma_start(
                        out=yt_part[j, i * 256:(i + 1) * 256, :].rearrange(
                            "(s p) q -> p s q", s=2
                        ),
                        in_=ysb[:],
                    )
                    nc.gpsimd.collective_compute(
                        "ReduceScatter",
                        ALU.add,
                        replica_groups=RG,
                        ins=[yt_part[j, i * 256:(i + 1) * 256, :].opt()],
                        outs=[yt_rs[j, i * 128:(i + 1) * 128, :].opt()],
                    )
                    nc.sync.dma_start(
                        out=yt_d[i * 128:(i + 1) * 128, jr],
                        in_=yt_rs[j, i * 128:(i + 1) * 128, :],
                    )

    nc.finalize()
    return nc


def _prep_inputs(x, Wq, Wk, Wv, Wo, bo):
    """Build the 8 per-core input maps (host-side layout prep only)."""
    import ml_dtypes

    scale = 1.0 / np.sqrt(np.float32(HD))
    kr = np.arange(128, dtype=np.float32)[:, None]
    qc = np.arange(128, dtype=np.float32)[None, :]
    m0 = (qc >= kr).astype(ml_dtypes.bfloat16)

    in_maps = []
    for c in range(NCORES):
        b, g = c // 2, c % 2
        hs = slice(g * 8, (g + 1) * 8)
        xt = np.ascontiguousarray(x[b].T).astype(ml_dtypes.bfloat16)
        wq = np.ascontiguousarray(Wq[hs].reshape(512, D).T * scale).astype(ml_dtypes.bfloat16)
        wk = np.ascontiguousarray(Wk[hs].reshape(512, D).T).astype(ml_dtypes.bfloat16)
        wv = np.ascontiguousarray(Wv[hs].reshape(512, D).T).astype(ml_dtypes.bfloat16)
        wo = np.ascontiguousarray(Wo[:, g * 512:(g + 1) * 512].T).astype(ml_dtypes.bfloat16)
        if g == 0:
            bias = np.ascontiguousarray(bo.reshape(8, 128).T)
        else:
            bias = np.zeros((128, 8), np.float32)
        in_maps.append(
            {"xt": xt, "wq": wq, "wk": wk, "wv": wv, "wo": wo, "bias": bias, "mask": m0}
        )
    return in_maps


def _run(inputs, trace=False, trace_cores=None):
    from concourse.bass_utils import run_bass_kernel_spmd

    if "nc" not in _CACHE:
        _CACHE["nc"] = _build_nc()
    nc = _CACHE["nc"]
    in_maps = _prep_inputs(
        inputs["x"], inputs["Wq"], inputs["Wk"], inputs["Wv"], inputs["Wo"], inputs["bo"]
    )
    r = run_bass_kernel_spmd(
        nc, in_maps, list(range(NCORES)), trace=trace, trace_cores=trace_cores
    )
    y = np.empty((B, T, D), np.float32)
    for b in range(B):
        yt = np.concatenate([r.results[2 * b]["yt"], r.results[2 * b + 1]["yt"]], axis=0)
        y[b] = yt.T.astype(np.float32)
    return y, r


def kernel(**inputs):
    y, _ = _run(inputs, trace=False)
    return y


# revision 11
# speedup vs baseline: 1.1937x; 1.1651x over previous
"""Trainium2 Bass kernel for causal multi-head attention (B=4, T=2048, D=1024, H=16).

Sharding: 8 cores = 4 batches x 2 head-groups (8 heads each).
Per core pipeline (Tile framework, single SPMD program):
  phase 1: Q/K projections into transposed per-head-pair layout QT/KT [128=2*64, 512]
           per t-range, V projection into [t, 8*65] layout (65th col = ones).
  phase 2: per (q-range of 512, head-pair): causal flash attention in transposed
           layout: ST[k,q] = KT^T @ QT (row-packed matmul pair), PT = exp(ST) (ACT),
           triangle mask on diagonal 128x128 sub-blocks (DVE), OT += [V|1]^T @ PT,
           normalize via fast reciprocal of the rowsum row.
  phase 3: output projection YT[dout, t] = Wo_gT^T @ OT + bias, 4-chunk bf16
           ReduceScatter across the batch pair (chunk i = d-blocks (i, i+4)).
Projection work for range j+1 and output-projection work for range j-1 are
emitted interleaved into attention(j)'s loop so the tensor queue can fill the
ACT-paced gaps; per-range tiles keep the dependence graph free of false WARs.
Host: transpose/slice weights, assemble [B, T, D] from per-core [512, T] halves.
"""

import numpy as np

B, T, D, H, HD = 4, 2048, 1024, 16, 64
NCORES = 8
NP = 4          # head pairs per core
NJ = 4          # q-ranges of 512
QW = 512
TB = T // 128   # 16

_CACHE = {}


def _build_nc():
    import concourse.mybir as mybir
    import concourse.tile as tile
    from concourse import bacc

    F32 = mybir.dt.float32
    BF16 = mybir.dt.bfloat16
    AF = mybir.ActivationFunctionType
    ALU = mybir.AluOpType

    nc = bacc.Bacc(None, target_bir_lowering=False)
    xt_d = nc.declare_dram_parameter("xt", [D, T], BF16, isOutput=False)
    wq_d = nc.declare_dram_parameter("wq", [D, 512], BF16, isOutput=False)
    wk_d = nc.declare_dram_parameter("wk", [D, 512], BF16, isOutput=False)
    wv_d = nc.declare_dram_parameter("wv", [D, 512], BF16, isOutput=False)
    wo_d = nc.declare_dram_parameter("wo", [512, D], BF16, isOutput=False)
    bias_d = nc.declare_dram_parameter("bias", [128, 8], F32, isOutput=False)
    mask_d = nc.declare_dram_parameter("mask", [128, 128], BF16, isOutput=False)
    yt_d = nc.declare_dram_parameter("yt", [512, T], BF16, isOutput=True)

    RG = [[0, 1], [2, 3], [4, 5], [6, 7]]

    with tile.TileContext(nc) as tc:
        with (
            tc.tile_pool(name="persist", bufs=1) as pers,
            tc.tile_pool(name="work", bufs=1) as work,
            tc.tile_pool(name="dram", bufs=1, space="DRAM") as dram,
            tc.tile_pool(name="psum", bufs=1, space="PSUM") as psum,
        ):
            # per-q-range tiles (separate objects -> no false WAR between ranges)
            qt = [pers.tile([128, NP, QW], BF16, name=f"qt{_j}", tag=f"qt{_j}") for _j in range(NJ)]
            kt = [pers.tile([128, NP, QW], BF16, name=f"kt{_j}", tag=f"kt{_j}") for _j in range(NJ)]
            v = [pers.tile([128, 4, 8 * 65], BF16, name=f"v{_j}", tag=f"v{_j}") for _j in range(NJ)]
            ot = [pers.tile([128, NP, QW], BF16, name=f"ot{_j}", tag=f"ot{_j}") for _j in range(NJ)]
            m0 = pers.tile([128, 128], BF16)
            wo = pers.tile([128, 4, D], BF16)
            bias = pers.tile([128, 8], F32)
            wq = pers.tile([128, 8, 512], BF16)
            wk = pers.tile([128, 8, 512], BF16)
            wv = pers.tile([128, 8, 512], BF16)

            def load_xs(j):
                xs = work.tile([128, 8, QW], BF16, tag="xs", bufs=2)
                nc.sync.dma_start(
                    out=xs[:],
                    in_=xt_d[:, j * QW:(j + 1) * QW].rearrange("(c p) t -> p c t", p=128),
                )
                return xs

            nc.sync.dma_start(out=m0[:], in_=mask_d[:])
            nc.sync.dma_start(out=wq[:], in_=wq_d.rearrange("(c p) n -> p c n", p=128))
            xs_tiles = {0: load_xs(0)}
            nc.sync.dma_start(out=wk[:], in_=wk_d.rearrange("(c p) n -> p c n", p=128))
            nc.sync.dma_start(out=wv[:], in_=wv_d.rearrange("(c p) n -> p c n", p=128))
            nc.sync.dma_start(out=wo[:], in_=wo_d.rearrange("(c p) n -> p c n", p=128))
            nc.sync.dma_start(out=bias[:], in_=bias_d[:])

            yt_part = dram.tile([NJ, 1024, QW], BF16)
            yt_rs = dram.tile([NJ, 512, QW], BF16)

            def proj_qk_group(j, p, w_sb, dst):
                xs = xs_tiles[j]
                acc = psum.tile([128, QW], F32, tag="small", bufs=2)
                for c in range(8):
                    nc.tensor.matmul(
                        acc[:], w_sb[:, c, p * 128:(p + 1) * 128], xs[:, c, :],
                        start=(c == 0), stop=(c == 7),
                    )
                nc.vector.tensor_copy(dst[j][:, p, :], acc[:])

            def proj_v_group(j, sub):
                xs = xs_tiles[j]
                acc = psum.tile([128, QW], F32, tag="small", bufs=2)
                for c in range(8):
                    nc.tensor.matmul(
                        acc[:], xs[:, c, sub * 128:(sub + 1) * 128], wv[:, c, :],
                        start=(c == 0), stop=(c == 7),
                    )
                vblk = v[j][:, sub, :].rearrange("p (h c) -> p h c", c=65)
                nc.vector.tensor_copy(
                    vblk[:, :, 0:64], acc[:].rearrange("p (h c) -> p h c", c=64)
                )
                nc.gpsimd.memset(vblk[:, :, 64:65], 1.0)

            def proj_fillers(j):
                yield lambda: xs_tiles.__setitem__(j, load_xs(j))
                for p in range(NP):
                    for w_sb, dst in ((wq, qt), (wk, kt)):
                        yield lambda j=j, p=p, w_sb=w_sb, dst=dst: proj_qk_group(j, p, w_sb, dst)
                for sub in range(4):
                    yield lambda j=j, sub=sub: proj_v_group(j, sub)

            def y_chunk(j, i):
                jr = slice(j * QW, (j + 1) * QW)
                ysb = work.tile([128, 2, QW], BF16, tag="ysb", bufs=3)
                for s, n in enumerate((i, i + 4)):
                    yps = psum.tile([128, QW], F32, tag="small", bufs=2)
                    for c in range(4):
                        nc.tensor.matmul(
                            yps[:], wo[:, c, n * 128:(n + 1) * 128], ot[j][:, c, :],
                            start=(c == 0), stop=(c == 3),
                        )
                    nc.vector.tensor_scalar_add(ysb[:, s, :], yps[:], bias[:, n:n + 1])
                nc.sync.dma_start(
                    out=yt_part[j, i * 256:(i + 1) * 256, :].rearrange(
                        "(s p) q -> p s q", s=2
                    ),
                    in_=ysb[:],
                )
                nc.gpsimd.collective_compute(
                    "ReduceScatter", ALU.add, replica_groups=RG,
                    ins=[yt_part[j, i * 256:(i + 1) * 256, :].opt()],
                    outs=[yt_rs[j, i * 128:(i + 1) * 128, :].opt()],
                )
                nc.sync.dma_start(
                    out=yt_d[i * 128:(i + 1) * 128, jr],
                    in_=yt_rs[j, i * 128:(i + 1) * 128, :],
                )

            def y_fillers(j):
                for i in range(4):
                    yield lambda j=j, i=i: y_chunk(j, i)

            def attention(j):
                """phase 2 for range j, with filler groups for Y(j-1)/proj(j+1)."""
                pf = list(proj_fillers(j + 1)) if j + 1 < NJ else []
                yf = list(y_fillers(j - 1)) if j > 0 else []
                fillers = pf[:1]  # xs prefetch first
                pf = pf[1:]
                # round-robin the remaining proj groups with Y(j-1) chunks
                k = max(len(pf), len(yf))
                for idx in range(k):
                    if idx < len(pf):
                        fillers.append(pf[idx])
                    if idx < len(yf):
                        fillers.append(yf[idx])
                n_iters = NP * (4 * j + 4)
                emitted = 0
                it = 0
                for p in range(NP):
                    hA, hB = 2 * p, 2 * p + 1
                    o_A = psum.tile([65, QW], F32, tag="o", bufs=2)
                    o_B = psum.tile([65, QW], F32, tag="o", bufs=2)
                    nkb = 4 * j + 4
                    for kb in range(nkb):
                        o = kb - 4 * j  # diagonal offset; < 0 means full block
                        lo = 128 * o if o > 0 else 0
                        st = psum.tile([128, 1024], F32, tag="st", bufs=2)
                        kj, kb_l = kb // 4, kb % 4
                        kcols = slice(kb_l * 128, (kb_l + 1) * 128)
                        qcols = slice(lo, QW)
                        nc.tensor.matmul(
                            st[:, lo:QW],
                            kt[kj][0:64, p, kcols],
                            qt[j][0:64, p, qcols],
                            start=True, stop=True, tile_position=(0, 0),
                        )
                        nc.tensor.matmul(
                            st[:, QW + lo:2 * QW],
                            kt[kj][64:128, p, kcols],
                            qt[j][64:128, p, qcols],
                            start=True, stop=True, tile_position=(64, 0),
                        )
                        pt = work.tile([128, 1024], BF16, tag="pt", bufs=4)
                        nc.scalar.activation(
                            pt[:].rearrange("p (h q) -> p h q", h=2)[:, :, lo:QW],
                            st[:].rearrange("p (h q) -> p h q", h=2)[:, :, lo:QW],
                            AF.Exp,
                        )
                        if o >= 0:
                            nc.vector.tensor_mul(
                                pt[:, lo:lo + 128], pt[:, lo:lo + 128], m0[:]
                            )
                            nc.vector.tensor_mul(
                                pt[:, QW + lo:QW + lo + 128],
                                pt[:, QW + lo:QW + lo + 128],
                                m0[:],
                            )
                        nc.tensor.matmul(
                            o_A[:, lo:QW],
                            v[kj][:, kb_l, hA * 65:(hA + 1) * 65],
                            pt[:, lo:QW],
                            start=(kb == 0), stop=(kb == nkb - 1),
                        )
                        nc.tensor.matmul(
                            o_B[:, lo:QW],
                            v[kj][:, kb_l, hB * 65:(hB + 1) * 65],
                            pt[:, QW + lo:2 * QW],
                            start=(kb == 0), stop=(kb == nkb - 1),
                        )
                        it += 1
                        while fillers and emitted < (it * len(fillers)) // n_iters:
                            fillers[emitted]()
                            emitted += 1
                    # stage o out of PSUM quickly, then normalize from SBUF
                    ocp = work.tile([65, 1024], F32, tag="ocp", bufs=3)
                    nc.vector.tensor_copy(ocp[:, 0:QW], o_A[:])
                    nc.vector.tensor_copy(ocp[:, QW:1024], o_B[:])
                    rsum = work.tile([1, 1024], F32, tag="rsum", bufs=2)
                    nc.vector.tensor_copy(rsum[:], ocp[64:65, :])
                    rec = work.tile([1, 1024], F32, tag="rec", bufs=2)
                    nc.vector.reciprocal_approx_fast(rec[:], rsum[:])
                    bc = work.tile([64, 1024], F32, tag="bc", bufs=2)
                    nc.gpsimd.partition_broadcast(bc[:, 0:QW], rec[:, 0:QW], channels=64)
                    nc.gpsimd.partition_broadcast(bc[:, QW:1024], rec[:, QW:1024], channels=64)
                    nc.vector.tensor_mul(ot[j][0:64, p, :], ocp[0:64, 0:QW], bc[:, 0:QW])
                    nc.vector.tensor_mul(ot[j][64:128, p, :], ocp[0:64, QW:1024], bc[:, QW:1024])
                for f in fillers[emitted:]:
                    f()

            # phase 1 for j=0 up front, then attention(j) with interleaved fillers
            for p in range(NP):
                for w_sb, dst in ((wq, qt), (wk, kt)):
                    proj_qk_group(0, p, w_sb, dst)
            for sub in range(4):
                proj_v_group(0, sub)

            for j in range(NJ):
                attention(j)
            for i in range(4):
                y_chunk(NJ - 1, i)

    nc.finalize()
    return nc


def _prep_inputs(x, Wq, Wk, Wv, Wo, bo):
    """Build the 8 per-core input maps (host-side layout prep only)."""
    import ml_dtypes

    scale = 1.0 / np.sqrt(np.float32(HD))
    kr = np.arange(128, dtype=np.float32)[:, None]
    qc = np.arange(128, dtype=np.float32)[None, :]
    m0 = (qc >= kr).astype(ml_dtypes.bfloat16)

    in_maps = []
    for c in range(NCORES):
        b, g = c // 2, c % 2
        hs = slice(g * 8, (g + 1) * 8)
        xt = np.ascontiguousarray(x[b].T).astype(ml_dtypes.bfloat16)
        wq = np.ascontiguousarray(Wq[hs].reshape(512, D).T * scale).astype(ml_dtypes.bfloat16)
        wk = np.ascontiguousarray(Wk[hs].reshape(512, D).T).astype(ml_dtypes.bfloat16)
        wv = np.ascontiguousarray(Wv[hs].reshape(512, D).T).astype(ml_dtypes.bfloat16)
        wo = np.ascontiguousarray(Wo[:, g * 512:(g + 1) * 512].T).astype(ml_dtypes.bfloat16)
        if g == 0:
            bias = np.ascontiguousarray(bo.reshape(8, 128).T)
        else:
            bias = np.zeros((128, 8), np.float32)
        in_maps.append(
            {"xt": xt, "wq": wq, "wk": wk, "wv": wv, "wo": wo, "bias": bias, "mask": m0}
        )
    return in_maps


def _run(inputs, trace=False, trace_cores=None):
    from concourse.bass_utils import run_bass_kernel_spmd

    if "nc" not in _CACHE:
        _CACHE["nc"] = _build_nc()
    nc = _CACHE["nc"]
    in_maps = _prep_inputs(
        inputs["x"], inputs["Wq"], inputs["Wk"], inputs["Wv"], inputs["Wo"], inputs["bo"]
    )
    r = run_bass_kernel_spmd(
        nc, in_maps, list(range(NCORES)), trace=trace, trace_cores=trace_cores
    )
    y = np.empty((B, T, D), np.float32)
    for b in range(B):
        yt = np.concatenate([r.results[2 * b]["yt"], r.results[2 * b + 1]["yt"]], axis=0)
        y[b] = yt.T.astype(np.float32)
    return y, r


def kernel(**inputs):
    y, _ = _run(inputs, trace=False)
    return y


# revision 14
# speedup vs baseline: 1.2990x; 1.0882x over previous
"""Trainium2 Bass kernel for causal multi-head attention (B=4, T=2048, D=1024, H=16).

Sharding: 8 cores = 4 batches x 2 head-groups (8 heads each).
Per core pipeline (Tile framework, single SPMD program):
  phase 1: Q/K projections into transposed per-head-pair layout QT/KT [128=2*64, 512]
           per t-range, V projection into [t, 8*65] layout (65th col = ones).
  phase 2: per (q-range of 512, head-pair): causal flash attention in transposed
           layout: ST[k,q] = KT^T @ QT (row-packed matmul pair), PT = exp(ST) (ACT),
           triangle mask on diagonal 128x128 sub-blocks (DVE), OT += [V|1]^T @ PT,
           normalize via fast reciprocal of the rowsum row.
  phase 3: output projection YT[dout, t] = Wo_gT^T @ OT + bias, 4-chunk bf16
           ReduceScatter across the batch pair (chunk i = d-blocks (i, i+4)).
Projection work for range j+1 and output-projection work for range j-1 are
emitted interleaved into attention(j)'s loop so the tensor queue can fill the
ACT-paced gaps; per-range tiles keep the dependence graph free of false WARs.
Host: transpose/slice weights, assemble [B, T, D] from per-core [512, T] halves.
"""

import numpy as np

B, T, D, H, HD = 4, 2048, 1024, 16, 64
NCORES = 8
NP = 4          # head pairs per core
NJ = 4          # q-ranges of 512
QW = 512
TB = T // 128   # 16

_CACHE = {}


def _build_nc():
    import concourse.mybir as mybir
    import concourse.tile as tile
    from concourse import bacc

    F32 = mybir.dt.float32
    BF16 = mybir.dt.bfloat16
    AF = mybir.ActivationFunctionType
    ALU = mybir.AluOpType

    nc = bacc.Bacc(None, target_bir_lowering=False)
    xt_d = nc.declare_dram_parameter("xt", [D, T], BF16, isOutput=False)
    wq_d = nc.declare_dram_parameter("wq", [D, 512], BF16, isOutput=False)
    wk_d = nc.declare_dram_parameter("wk", [D, 512], BF16, isOutput=False)
    wv_d = nc.declare_dram_parameter("wv", [D, 512], BF16, isOutput=False)
    wo_d = nc.declare_dram_parameter("wo", [512, D], BF16, isOutput=False)
    bias_d = nc.declare_dram_parameter("bias", [128, 8], F32, isOutput=False)
    mask_d = nc.declare_dram_parameter("mask", [128, 128], BF16, isOutput=False)
    yt_d = nc.declare_dram_parameter("yt", [512, T], BF16, isOutput=True)

    RG = [[0, 1], [2, 3], [4, 5], [6, 7]]

    with tile.TileContext(nc) as tc:
        with (
            tc.tile_pool(name="persist", bufs=1) as pers,
            tc.tile_pool(name="work", bufs=1) as work,
            tc.tile_pool(name="dram", bufs=1, space="DRAM") as dram,
            tc.tile_pool(name="psum", bufs=1, space="PSUM") as psum,
        ):
            # per-q-range tiles (separate objects -> no false WAR between ranges)
            qt = [pers.tile([128, NP, QW], BF16, name=f"qt{_j}", tag=f"qt{_j}") for _j in range(NJ)]
            kt = [pers.tile([128, NP, QW], BF16, name=f"kt{_j}", tag=f"kt{_j}") for _j in range(NJ)]
            v = [pers.tile([128, 4, 8 * 65], BF16, name=f"v{_j}", tag=f"v{_j}") for _j in range(NJ)]
            ot = [pers.tile([128, NP, QW], BF16, name=f"ot{_j}", tag=f"ot{_j}") for _j in range(NJ)]
            m0 = pers.tile([128, 128], BF16)
            wo = pers.tile([128, 4, D], BF16)
            bias = pers.tile([128, 8], F32)
            wq = pers.tile([128, 8, 512], BF16)
            wk = pers.tile([128, 8, 512], BF16)
            wv = pers.tile([128, 8, 512], BF16)

            def load_xs(j):
                xs = work.tile([128, 8, QW], BF16, tag="xs", bufs=2)
                nc.sync.dma_start(
                    out=xs[:],
                    in_=xt_d[:, j * QW:(j + 1) * QW].rearrange("(c p) t -> p c t", p=128),
                )
                return xs

            # split the startup-critical loads across DMA queues for parallelism
            wq_r = wq_d.rearrange("(c p) n -> p c n", p=128)
            xt_r = xt_d[:, 0:QW].rearrange("(c p) t -> p c t", p=128)
            xs0 = work.tile([128, 8, QW], BF16, tag="xs", bufs=2)
            for h in range(4):
                cs = slice(2 * h, 2 * h + 2)
                nc.sync.dma_start(out=wq[:, cs, :], in_=wq_r[:, cs, :])
                nc.sync.dma_start(out=xs0[:, cs, :], in_=xt_r[:, cs, :])
            xs_tiles = {0: xs0}
            nc.sync.dma_start(out=wk[:], in_=wk_d.rearrange("(c p) n -> p c n", p=128))
            nc.sync.dma_start(out=m0[:], in_=mask_d[:])
            nc.sync.dma_start(out=wv[:], in_=wv_d.rearrange("(c p) n -> p c n", p=128))
            nc.sync.dma_start(out=wo[:], in_=wo_d.rearrange("(c p) n -> p c n", p=128))
            nc.sync.dma_start(out=bias[:], in_=bias_d[:])

            yt_part = dram.tile([NJ, 1024, QW], BF16)
            yt_rs = dram.tile([NJ, 512, QW], BF16)

            def proj_qk_group(j, p, w_sb, dst):
                xs = xs_tiles[j]
                acc = psum.tile([128, QW], F32, tag="small", bufs=2)
                for c in range(8):
                    nc.tensor.matmul(
                        acc[:], w_sb[:, c, p * 128:(p + 1) * 128], xs[:, c, :],
                        start=(c == 0), stop=(c == 7),
                    )
                nc.vector.tensor_copy(dst[j][:, p, :], acc[:])

            def proj_v_group(j, sub):
                xs = xs_tiles[j]
                acc = psum.tile([128, QW], F32, tag="small", bufs=2)
                for c in range(8):
                    nc.tensor.matmul(
                        acc[:], xs[:, c, sub * 128:(sub + 1) * 128], wv[:, c, :],
                        start=(c == 0), stop=(c == 7),
                    )
                vblk = v[j][:, sub, :].rearrange("p (h c) -> p h c", c=65)
                nc.vector.tensor_copy(
                    vblk[:, :, 0:64], acc[:].rearrange("p (h c) -> p h c", c=64)
                )
                nc.gpsimd.memset(vblk[:, :, 64:65], 1.0)

            def proj_fillers(j):
                yield lambda: xs_tiles.__setitem__(j, load_xs(j))
                for p in range(NP):
                    for w_sb, dst in ((wq, qt), (wk, kt)):
                        yield lambda j=j, p=p, w_sb=w_sb, dst=dst: proj_qk_group(j, p, w_sb, dst)
                for sub in range(4):
                    yield lambda j=j, sub=sub: proj_v_group(j, sub)

            def y_block(j, n):
                ysb = work.tile([128, QW], BF16, tag="ysb", bufs=3)
                yps = psum.tile([128, QW], F32, tag="small", bufs=2)
                for c in range(4):
                    nc.tensor.matmul(
                        yps[:], wo[:, c, n * 128:(n + 1) * 128], ot[j][:, c, :],
                        start=(c == 0), stop=(c == 3),
                    )
                nc.vector.tensor_scalar_add(ysb[:], yps[:], bias[:, n:n + 1])
                nc.sync.dma_start(out=yt_part[j, n * 128:(n + 1) * 128, :], in_=ysb[:])

            def y_finalize(j):
                # one RS per range: the ~7us fixed cost per collective dominates,
                # so a single big call beats chunked ones
                jr = slice(j * QW, (j + 1) * QW)
                nc.gpsimd.collective_compute(
                    "ReduceScatter", ALU.add, replica_groups=RG,
                    ins=[yt_part[j].opt()],
                    outs=[yt_rs[j].opt()],
                )
                nc.sync.dma_start(out=yt_d[:, jr], in_=yt_rs[j])

            def y_fillers(j):
                for n in range(8):
                    yield lambda j=j, n=n: y_block(j, n)
                yield lambda j=j: y_finalize(j)

            def attention(j):
                """phase 2 for range j, with filler groups for Y(j-1)/proj(j+1)."""
                pf = list(proj_fillers(j + 1)) if j + 1 < NJ else []
                yf = list(y_fillers(j - 1)) if j > 0 else []
                fillers = pf[:1]  # xs prefetch first
                pf = pf[1:]
                # round-robin the remaining proj groups with Y(j-1) chunks
                k = max(len(pf), len(yf))
                for idx in range(k):
                    if idx < len(pf):
                        fillers.append(pf[idx])
                    if idx < len(yf):
                        fillers.append(yf[idx])
                n_iters = NP * (4 * j + 4)
                emitted = 0
                it = 0
                for p in range(NP):
                    hA, hB = 2 * p, 2 * p + 1
                    o_A = psum.tile([65, QW], F32, tag="o", bufs=2)
                    o_B = psum.tile([65, QW], F32, tag="o", bufs=2)
                    nkb = 4 * j + 4
                    for kb in range(nkb):
                        o = kb - 4 * j  # diagonal offset; < 0 means full block
                        lo = 128 * o if o > 0 else 0
                        st = psum.tile([128, 1024], F32, tag="st", bufs=2)
                        kj, kb_l = kb // 4, kb % 4
                        kcols = slice(kb_l * 128, (kb_l + 1) * 128)
                        qcols = slice(lo, QW)
                        nc.tensor.matmul(
                            st[:, lo:QW],
                            kt[kj][0:64, p, kcols],
                            qt[j][0:64, p, qcols],
                            start=True, stop=True, tile_position=(0, 0),
                        )
                        nc.tensor.matmul(
                            st[:, QW + lo:2 * QW],
                            kt[kj][64:128, p, kcols],
                            qt[j][64:128, p, qcols],
                            start=True, stop=True, tile_position=(64, 0),
                        )
                        pt = work.tile([128, 1024], BF16, tag="pt", bufs=4)
                        nc.scalar.activation(
                            pt[:].rearrange("p (h q) -> p h q", h=2)[:, :, lo:QW],
                            st[:].rearrange("p (h q) -> p h q", h=2)[:, :, lo:QW],
                            AF.Exp,
                        )
                        if o >= 0:
                            nc.vector.tensor_mul(
                                pt[:, lo:lo + 128], pt[:, lo:lo + 128], m0[:]
                            )
                            nc.vector.tensor_mul(
                                pt[:, QW + lo:QW + lo + 128],
                                pt[:, QW + lo:QW + lo + 128],
                                m0[:],
                            )
                        nc.tensor.matmul(
                            o_A[:, lo:QW],
                            v[kj][:, kb_l, hA * 65:(hA + 1) * 65],
                            pt[:, lo:QW],
                            start=(kb == 0), stop=(kb == nkb - 1),
                        )
                        nc.tensor.matmul(
                            o_B[:, lo:QW],
                            v[kj][:, kb_l, hB * 65:(hB + 1) * 65],
                            pt[:, QW + lo:2 * QW],
                            start=(kb == 0), stop=(kb == nkb - 1),
                        )
                        it += 1
                        while fillers and emitted < (it * len(fillers)) // n_iters:
                            fillers[emitted]()
                            emitted += 1
                    # stage o out of PSUM quickly, then normalize from SBUF
                    ocp = work.tile([65, 1024], F32, tag="ocp", bufs=3)
                    nc.vector.tensor_copy(ocp[:, 0:QW], o_A[:])
                    nc.vector.tensor_copy(ocp[:, QW:1024], o_B[:])
                    rsum = work.tile([1, 1024], F32, tag="rsum", bufs=2)
                    nc.vector.tensor_copy(rsum[:], ocp[64:65, :])
                    rec = work.tile([1, 1024], F32, tag="rec", bufs=2)
                    nc.vector.reciprocal_approx_fast(rec[:], rsum[:])
                    bc = work.tile([64, 1024], F32, tag="bc", bufs=2)
                    nc.gpsimd.partition_broadcast(bc[:, 0:QW], rec[:, 0:QW], channels=64)
                    nc.gpsimd.partition_broadcast(bc[:, QW:1024], rec[:, QW:1024], channels=64)
                    nc.vector.tensor_mul(ot[j][0:64, p, :], ocp[0:64, 0:QW], bc[:, 0:QW])
                    nc.vector.tensor_mul(ot[j][64:128, p, :], ocp[0:64, QW:1024], bc[:, QW:1024])
                for f in fillers[emitted:]:
                    f()

            # phase 1 for j=0 up front, then attention(j) with interleaved fillers
            for p in range(NP):
                for w_sb, dst in ((wq, qt), (wk, kt)):
                    proj_qk_group(0, p, w_sb, dst)
            for sub in range(4):
                proj_v_group(0, sub)

            for j in range(NJ):
                attention(j)
            for n in range(8):
                y_block(NJ - 1, n)
            y_finalize(NJ - 1)

    nc.finalize()
    return nc


def _prep_inputs(x, Wq, Wk, Wv, Wo, bo):
    """Build the 8 per-core input maps (host-side layout prep only)."""
    import ml_dtypes

    scale = 1.0 / np.sqrt(np.float32(HD))
    kr = np.arange(128, dtype=np.float32)[:, None]
    qc = np.arange(128, dtype=np.float32)[None, :]
    m0 = (qc >= kr).astype(ml_dtypes.bfloat16)

    in_maps = []
    for c in range(NCORES):
        b, g = c // 2, c % 2
        hs = slice(g * 8, (g + 1) * 8)
        xt = np.ascontiguousarray(x[b].T).astype(ml_dtypes.bfloat16)
        wq = np.ascontiguousarray(Wq[hs].reshape(512, D).T * scale).astype(ml_dtypes.bfloat16)
        wk = np.ascontiguousarray(Wk[hs].reshape(512, D).T).astype(ml_dtypes.bfloat16)
        wv = np.ascontiguousarray(Wv[hs].reshape(512, D).T).astype(ml_dtypes.bfloat16)
        wo = np.ascontiguousarray(Wo[:, g * 512:(g + 1) * 512].T).astype(ml_dtypes.bfloat16)
        if g == 0:
            bias = np.ascontiguousarray(bo.reshape(8, 128).T)
        else:
            bias = np.zeros((128, 8), np.float32)
        in_maps.append(
            {"xt": xt, "wq": wq, "wk": wk, "wv": wv, "wo": wo, "bias": bias, "mask": m0}
        )
    return in_maps


def _run(inputs, trace=False, trace_cores=None):
    from concourse.bass_utils import run_bass_kernel_spmd

    if "nc" not in _CACHE:
        _CACHE["nc"] = _build_nc()
    nc = _CACHE["nc"]
    in_maps = _prep_inputs(
        inputs["x"], inputs["Wq"], inputs["Wk"], inputs["Wv"], inputs["Wo"], inputs["bo"]
    )
    r = run_bass_kernel_spmd(
        nc, in_maps, list(range(NCORES)), trace=trace, trace_cores=trace_cores
    )
    y = np.empty((B, T, D), np.float32)
    for b in range(B):
        yt = np.concatenate([r.results[2 * b]["yt"], r.results[2 * b + 1]["yt"]], axis=0)
        y[b] = yt.T.astype(np.float32)
    return y, r


def kernel(**inputs):
    y, _ = _run(inputs, trace=False)
    return y
